# revision 7
# baseline (speedup 1.0000x reference)
"""Trainium2 Bass kernel for a bidirectional cross-attention layer.

Per batch sample (one NeuronCore each, 8 samples / 8 cores):
    e  = seq_1 @ seq_2^T                     [L, L]
    P  = exp(e)            (no max-subtraction: |e| <~ 70 << fp32 overflow)
    seq_1_hat = diag(1/rowsum(P)) @ P   @ seq_2
    seq_2_hat = diag(1/colsum(P)) @ P^T @ seq_1

Single exp pass: P is materialized once in bf16; the transposed orientation
P2 = P^T needed by the seq_1_hat accumulation comes from the DMA XBAR
transpose engine instead of recomputing scores + re-exponentiating (which
made the scalar engine the bottleneck). Matmul operands are fp16 (scores)
and bf16 (P, range needs f32 exponent); matmul outputs are 512-wide (one
PSUM bank). Row/col sums ride tensor_scalar accum_out on DVE (has 2x/4x
modes; tensor_reduce has none). Inputs load as two parallel f32 DMAs
(sync + scalar queues), cast to fp16 on ACT/DVE, then XBAR-transposed.
"""

import os

os.environ.setdefault("MYCRO_LOCAL_CACHE", "1")

import numpy as np

import concourse.mybir as mybir
from concourse import bacc
from concourse.bass_utils import run_bass_kernel_spmd
from concourse.tile import TileContext

B, L, D = 8, 2048, 128
NBLK = L // 128  # 16 blocks of 128

F32 = mybir.dt.float32
F16 = mybir.dt.float16
BF16 = mybir.dt.bfloat16
AF = mybir.ActivationFunctionType
ALU = mybir.AluOpType
AX = mybir.AxisListType


def _build():
    nc = bacc.Bacc(
        "TRN2", target_bir_lowering=False, debug=False, enable_asserts=False
    )
    s1 = nc.dram_tensor("seq_1", [L, D], F32, kind="ExternalInput").ap()
    s2 = nc.dram_tensor("seq_2", [L, D], F32, kind="ExternalInput").ap()
    o1 = nc.dram_tensor("out1", [L, D], BF16, kind="ExternalOutput").ap()
    o2 = nc.dram_tensor("out2", [L, D], BF16, kind="ExternalOutput").ap()

    with TileContext(nc) as tc:
        with tc.tile_pool(name="big", bufs=1) as big:
            s1f = big.tile([128, L], F32, tag="s1f")   # staging f32
            s2f = big.tile([128, L], F32, tag="s2f")
            s1h = big.tile([128, L], F16, tag="s1h")   # [i%128, blk*128+d]
            s2h = big.tile([128, L], F16, tag="s2h")
            s1t = big.tile([128, L], F16, tag="s1t")   # [d, i]
            s2t = big.tile([128, L], F16, tag="s2t")   # [d, j]
            P1 = big.tile([128, NBLK * L], BF16, tag="P1")  # [i%128, ib*L+j]
            P2 = big.tile([128, NBLK * L], BF16, tag="P2")  # [j%128, jb*L+i]
            o2h = big.tile([128, L], BF16, tag="o2h")  # bf16 copy of acc0
            o1h = big.tile([128, L], BF16, tag="o1h")
            o2s = big.tile([128, L], BF16, tag="o2s")  # xbar'd [j%128, jb*128+d]
            o1s = big.tile([128, L], BF16, tag="o1s")
            ob2 = big.tile([128, L], BF16, tag="ob2")  # scaled, store staging
            ob1 = big.tile([128, L], BF16, tag="ob1")
            scr = big.tile([128, L], BF16, tag="scr")  # accum_out dummy dest
            rowsum = big.tile([128, NBLK], F32, tag="rowsum")
            colsum = big.tile([128, NBLK], F32, tag="colsum")
            rrow = big.tile([128, NBLK], F32, tag="rrow")
            rcol = big.tile([128, NBLK], F32, tag="rcol")

            # ---- preload: parallel f32 loads, engine casts, XBAR -------
            nc.sync.dma_start(
                s2f.rearrange("p (b d) -> p b d", d=128),
                s2.rearrange("(b r) d -> r b d", r=128),
            )
            nc.scalar.dma_start(
                s1f.rearrange("p (b d) -> p b d", d=128),
                s1.rearrange("(b r) d -> r b d", r=128),
            )
            nc.scalar.copy(s2h, s2f)          # ACT cast f32 -> fp16
            nc.vector.tensor_copy(s1h, s1f)   # DVE cast in parallel
            for dst, src in ((s2t, s2h), (s1t, s1h)):
                nc.sync.dma_start_transpose(
                    dst.rearrange("p (b r) -> p b r", r=128), src
                )

            P2_3d = P2.rearrange("p (b i) -> p b i", i=L)

            # ---- phase A: P1 tiles, rowsum, o2T accumulation, P2 xbar ---
            with (
                tc.tile_pool(name="acc0p", bufs=1, space="PSUM") as acc0p,
                tc.tile_pool(name="ep", bufs=2, space="PSUM") as ep,
            ):
                acc0 = acc0p.tile([128, L], F32, tag="acc0")
                for b in range(NBLK):
                    bsl = slice(b * 128, (b + 1) * 128)
                    for h in range(2):
                        et = ep.tile([128, 1024], F32, tag="et")
                        for q in range(2):
                            # fp16 moving operand and PSUM bank cap: 512 wide
                            nc.tensor.matmul(
                                et[:, q * 512 : (q + 1) * 512],
                                lhsT=s1t[:, bsl],
                                rhs=s2t[:, h * 1024 + q * 512
                                        : h * 1024 + (q + 1) * 512],
                                start=True, stop=True,
                            )
                        nc.scalar.activation(
                            P1[:, b * L + h * 1024 : b * L + (h + 1) * 1024],
                            et, AF.Exp,
                        )
                        for q in range(2):
                            csl = slice(h * 1024 + q * 512,
                                        h * 1024 + (q + 1) * 512)
                            nc.tensor.matmul(
                                acc0[:, csl], lhsT=s1h[:, bsl],
                                rhs=P1[:, b * L + h * 1024 + q * 512
                                       : b * L + h * 1024 + (q + 1) * 512],
                                start=(b == 0), stop=(b == NBLK - 1),
                            )
                    # rowsum via tensor_scalar accum (DVE fast modes)
                    nc.vector.tensor_scalar(
                        scr, P1[:, b * L : (b + 1) * L], 1.0, None,
                        op0=ALU.mult, op1=ALU.add, accum_out=rowsum[:, b : b + 1],
                    )
                    nc.sync.dma_start_transpose(
                        P2_3d[:, :, b * 128 : (b + 1) * 128],
                        P1[:, b * L : (b + 1) * L],
                    )
                nc.vector.reciprocal(rrow, rowsum)
                # boundary: drain acc0 (split ACT/DVE) to free PSUM banks
                nc.scalar.copy(o2h[:, 0:1024], acc0[:, 0:1024])
                nc.vector.tensor_copy(o2h[:, 1024:2048], acc0[:, 1024:2048])

            # ---- phase B: o1T accumulation + colsum + o2 epilogue -------
            with tc.tile_pool(name="acc1p", bufs=1, space="PSUM") as acc1p:
                acc1 = acc1p.tile([128, L], F32, tag="acc1")
                nc.sync.dma_start_transpose(
                    o2s.rearrange("p (b d) -> p b d", d=128), o2h
                )
                for b in range(NBLK):
                    bsl = slice(b * 128, (b + 1) * 128)
                    for c in range(4):
                        isl = slice(c * 512, (c + 1) * 512)
                        nc.tensor.matmul(
                            acc1[:, isl], lhsT=s2h[:, bsl],
                            rhs=P2[:, b * L + c * 512 : b * L + (c + 1) * 512],
                            start=(b == 0), stop=(b == NBLK - 1),
                        )
                    nc.vector.tensor_scalar(
                        scr, P2[:, b * L : (b + 1) * L], 1.0, None,
                        op0=ALU.mult, op1=ALU.add, accum_out=colsum[:, b : b + 1],
                    )
                    nc.vector.reciprocal(
                        rcol[:, b : b + 1], colsum[:, b : b + 1]
                    )
                    nc.vector.tensor_scalar_mul(
                        ob2[:, bsl], o2s[:, bsl], rcol[:, b : b + 1]
                    )
                nc.sync.dma_start(
                    o2.rearrange("(b r) d -> r b d", r=128),
                    ob2.rearrange("p (b d) -> p b d", d=128),
                )

                # ---- tail: o1 epilogue, chunked halves for overlap ------
                for h in range(2):
                    isl = slice(h * 1024, (h + 1) * 1024)
                    if h == 0:
                        nc.scalar.copy(o1h[:, isl], acc1[:, isl])
                    else:
                        nc.vector.tensor_copy(o1h[:, isl], acc1[:, isl])
                for h in range(2):
                    isl = slice(h * 1024, (h + 1) * 1024)
                    nc.sync.dma_start_transpose(
                        o1s.rearrange("p (b d) -> p b d", d=128)[
                            :, h * 8 : (h + 1) * 8, :
                        ],
                        o1h[:, isl],
                    )
                    for k in range(h * 8, (h + 1) * 8):
                        nc.vector.tensor_scalar_mul(
                            ob1[:, k * 128 : (k + 1) * 128],
                            o1s[:, k * 128 : (k + 1) * 128],
                            rrow[:, k : k + 1],
                        )
                    nc.scalar.dma_start(
                        o1.rearrange("(b r) d -> r b d", r=128)[
                            :, h * 8 : (h + 1) * 8, :
                        ],
                        ob1.rearrange("p (b d) -> p b d", d=128)[
                            :, h * 8 : (h + 1) * 8, :
                        ],
                    )

    nc.compile()
    return nc


_nc_cache = None


def _run(seq_1, seq_2, trace=False):
    global _nc_cache
    if _nc_cache is None:
        _nc_cache = _build()
    nc = _nc_cache
    seq_1 = np.ascontiguousarray(np.asarray(seq_1, dtype=np.float32))
    seq_2 = np.ascontiguousarray(np.asarray(seq_2, dtype=np.float32))
    in_maps = [{"seq_1": seq_1[b], "seq_2": seq_2[b]} for b in range(B)]
    res = run_bass_kernel_spmd(nc, in_maps, core_ids=list(range(B)), trace=trace)
    out1 = np.stack(
        [np.asarray(res.results[b]["out1"]).astype(np.float32) for b in range(B)]
    )
    out2 = np.stack(
        [np.asarray(res.results[b]["out2"]).astype(np.float32) for b in range(B)]
    )
    return (out1, out2), res


def kernel(seq_1, seq_2):
    return _run(seq_1, seq_2)[0]


# revision 8
# speedup vs baseline: 1.2238x; 1.2238x over previous
"""Trainium2 Bass kernel for a bidirectional cross-attention layer.

Per batch sample (one NeuronCore each, 8 samples / 8 cores):
    e  = seq_1 @ seq_2^T                     [L, L]
    P  = exp(e)            (no max-subtraction: |e| <~ 70 << fp32 overflow)
    seq_1_hat = diag(1/rowsum(P)) @ P   @ seq_2
    seq_2_hat = diag(1/colsum(P)) @ P^T @ seq_1

Single exp pass: P is materialized once in bf16; the transposed orientation
P2 = P^T needed by the seq_1_hat accumulation comes from the DMA XBAR
transpose engine instead of recomputing scores + re-exponentiating (which
made the scalar engine the bottleneck). Matmul operands are fp16 (scores)
and bf16 (P, range needs f32 exponent); matmul outputs are 512-wide (one
PSUM bank). Row/col sums ride tensor_scalar accum_out on DVE (has 2x/4x
modes; tensor_reduce has none). Inputs load as two parallel f32 DMAs
(sync + scalar queues), cast to fp16 on ACT/DVE, then XBAR-transposed.
"""

import os

os.environ.setdefault("MYCRO_LOCAL_CACHE", "1")

import numpy as np

import concourse.mybir as mybir
from concourse import bacc
from concourse.bass_utils import run_bass_kernel_spmd
from concourse.tile import TileContext

B, L, D = 8, 2048, 128
NBLK = L // 128  # 16 blocks of 128

F32 = mybir.dt.float32
F16 = mybir.dt.float16
BF16 = mybir.dt.bfloat16
AF = mybir.ActivationFunctionType
ALU = mybir.AluOpType
AX = mybir.AxisListType


def _build():
    nc = bacc.Bacc(
        "TRN2", target_bir_lowering=False, debug=False, enable_asserts=False
    )
    s1 = nc.dram_tensor("seq_1", [L, D], F32, kind="ExternalInput").ap()
    s2 = nc.dram_tensor("seq_2", [L, D], F32, kind="ExternalInput").ap()
    o1 = nc.dram_tensor("out1", [L, D], BF16, kind="ExternalOutput").ap()
    o2 = nc.dram_tensor("out2", [L, D], BF16, kind="ExternalOutput").ap()

    with TileContext(nc) as tc:
        with tc.tile_pool(name="big", bufs=1) as big:
            s1f = big.tile([128, L], F32, tag="s1f")   # staging f32
            s2f = big.tile([128, L], F32, tag="s2f")
            s1h = big.tile([128, L], F16, tag="s1h")   # [i%128, blk*128+d]
            s2h = big.tile([128, L], F16, tag="s2h")
            s1t = big.tile([128, L], F16, tag="s1t")   # [d, i]
            s2t = big.tile([128, L], F16, tag="s2t")   # [d, j]
            P1 = big.tile([128, NBLK * L], BF16, tag="P1")  # [i%128, ib*L+j]
            P2 = big.tile([128, NBLK * L], BF16, tag="P2")  # [j%128, jb*L+i]
            o2h = big.tile([128, L], BF16, tag="o2h")  # bf16 copy of acc0
            o1h = big.tile([128, L], BF16, tag="o1h")
            o2s = big.tile([128, L], BF16, tag="o2s")  # xbar'd [j%128, jb*128+d]
            o1s = big.tile([128, L], BF16, tag="o1s")
            ob2 = big.tile([128, L], BF16, tag="ob2")  # scaled, store staging
            ob1 = big.tile([128, L], BF16, tag="ob1")
            scr = big.tile([128, L], BF16, tag="scr")  # accum_out dummy dest
            rsum4 = big.tile([128, 2 * NBLK], F32, tag="rsum4")
            rowsum = big.tile([128, NBLK], F32, tag="rowsum")
            colsum = big.tile([128, NBLK], F32, tag="colsum")
            rrow = big.tile([128, NBLK], F32, tag="rrow")
            rcol = big.tile([128, NBLK], F32, tag="rcol")

            # ---- preload: parallel f32 loads, engine casts, XBAR -------
            nc.sync.dma_start(
                s2f.rearrange("p (b d) -> p b d", d=128),
                s2.rearrange("(b r) d -> r b d", r=128),
            )
            nc.scalar.dma_start(
                s1f.rearrange("p (b d) -> p b d", d=128),
                s1.rearrange("(b r) d -> r b d", r=128),
            )
            nc.scalar.copy(s2h, s2f)          # ACT cast f32 -> fp16
            nc.vector.tensor_copy(s1h, s1f)   # DVE cast in parallel
            for dst, src in ((s2t, s2h), (s1t, s1h)):
                nc.sync.dma_start_transpose(
                    dst.rearrange("p (b r) -> p b r", r=128), src
                )

            P2_3d = P2.rearrange("p (b i) -> p b i", i=L)

            # ---- phase A: P1 tiles, rowsum, o2T accumulation, P2 xbar ---
            with (
                tc.tile_pool(name="acc0p", bufs=1, space="PSUM") as acc0p,
                tc.tile_pool(name="ep", bufs=2, space="PSUM") as ep,
            ):
                acc0 = acc0p.tile([128, L], F32, tag="acc0")
                for b in range(NBLK):
                    bsl = slice(b * 128, (b + 1) * 128)
                    for h in range(2):
                        et = ep.tile([128, 1024], F32, tag="et")
                        for q in range(2):
                            # fp16 moving operand and PSUM bank cap: 512 wide
                            nc.tensor.matmul(
                                et[:, q * 512 : (q + 1) * 512],
                                lhsT=s1t[:, bsl],
                                rhs=s2t[:, h * 1024 + q * 512
                                        : h * 1024 + (q + 1) * 512],
                                start=True, stop=True,
                            )
                        nc.scalar.activation(
                            P1[:, b * L + h * 1024 : b * L + (h + 1) * 1024],
                            et, AF.Exp,
                            accum_out=rsum4[:, 2 * b + h : 2 * b + h + 1],
                        )
                        for q in range(2):
                            csl = slice(h * 1024 + q * 512,
                                        h * 1024 + (q + 1) * 512)
                            nc.tensor.matmul(
                                acc0[:, csl], lhsT=s1h[:, bsl],
                                rhs=P1[:, b * L + h * 1024 + q * 512
                                       : b * L + h * 1024 + (q + 1) * 512],
                                start=(b == 0), stop=(b == NBLK - 1),
                            )
                    nc.sync.dma_start_transpose(
                        P2_3d[:, :, b * 128 : (b + 1) * 128],
                        P1[:, b * L : (b + 1) * L],
                    )
                nc.vector.tensor_reduce(
                    rowsum,
                    rsum4.rearrange("p (b two) -> p b two", two=2),
                    axis=AX.X, op=ALU.add,
                )
                nc.vector.reciprocal(rrow, rowsum)
                # boundary: drain acc0 (split ACT/DVE) to free PSUM banks
                nc.scalar.copy(o2h[:, 0:1024], acc0[:, 0:1024])
                nc.vector.tensor_copy(o2h[:, 1024:2048], acc0[:, 1024:2048])

            # ---- phase B: o1T accumulation + colsum + o2 epilogue -------
            with tc.tile_pool(name="acc1p", bufs=1, space="PSUM") as acc1p:
                acc1 = acc1p.tile([128, L], F32, tag="acc1")
                nc.sync.dma_start_transpose(
                    o2s.rearrange("p (b d) -> p b d", d=128), o2h
                )
                for b in range(NBLK):
                    bsl = slice(b * 128, (b + 1) * 128)
                    if b % 2 == 0:
                        nc.vector.tensor_reduce(
                            colsum[:, b : b + 1],
                            P2[:, b * L : (b + 1) * L],
                            axis=AX.X, op=ALU.add,
                        )
                    else:
                        nc.scalar.activation(
                            scr, P2[:, b * L : (b + 1) * L], AF.Copy,
                            accum_out=colsum[:, b : b + 1],
                        )
                    for c in range(4):
                        isl = slice(c * 512, (c + 1) * 512)
                        nc.tensor.matmul(
                            acc1[:, isl], lhsT=s2h[:, bsl],
                            rhs=P2[:, b * L + c * 512 : b * L + (c + 1) * 512],
                            start=(b == 0), stop=(b == NBLK - 1),
                        )
                    nc.vector.reciprocal(
                        rcol[:, b : b + 1], colsum[:, b : b + 1]
                    )
                    nc.vector.tensor_scalar_mul(
                        ob2[:, bsl], o2s[:, bsl], rcol[:, b : b + 1]
                    )
                nc.sync.dma_start(
                    o2.rearrange("(b r) d -> r b d", r=128),
                    ob2.rearrange("p (b d) -> p b d", d=128),
                )

                # ---- tail: o1 epilogue, chunked halves for overlap ------
                for h in range(2):
                    isl = slice(h * 1024, (h + 1) * 1024)
                    if h == 0:
                        nc.scalar.copy(o1h[:, isl], acc1[:, isl])
                    else:
                        nc.vector.tensor_copy(o1h[:, isl], acc1[:, isl])
                for h in range(2):
                    isl = slice(h * 1024, (h + 1) * 1024)
                    nc.sync.dma_start_transpose(
                        o1s.rearrange("p (b d) -> p b d", d=128)[
                            :, h * 8 : (h + 1) * 8, :
                        ],
                        o1h[:, isl],
                    )
                    for k in range(h * 8, (h + 1) * 8):
                        nc.vector.tensor_scalar_mul(
                            ob1[:, k * 128 : (k + 1) * 128],
                            o1s[:, k * 128 : (k + 1) * 128],
                            rrow[:, k : k + 1],
                        )
                    nc.scalar.dma_start(
                        o1.rearrange("(b r) d -> r b d", r=128)[
                            :, h * 8 : (h + 1) * 8, :
                        ],
                        ob1.rearrange("p (b d) -> p b d", d=128)[
                            :, h * 8 : (h + 1) * 8, :
                        ],
                    )

    nc.compile()
    return nc


_nc_cache = None


def _run(seq_1, seq_2, trace=False):
    global _nc_cache
    if _nc_cache is None:
        _nc_cache = _build()
    nc = _nc_cache
    seq_1 = np.ascontiguousarray(np.asarray(seq_1, dtype=np.float32))
    seq_2 = np.ascontiguousarray(np.asarray(seq_2, dtype=np.float32))
    in_maps = [{"seq_1": seq_1[b], "seq_2": seq_2[b]} for b in range(B)]
    res = run_bass_kernel_spmd(nc, in_maps, core_ids=list(range(B)), trace=trace)
    out1 = np.stack(
        [np.asarray(res.results[b]["out1"]).astype(np.float32) for b in range(B)]
    )
    out2 = np.stack(
        [np.asarray(res.results[b]["out2"]).astype(np.float32) for b in range(B)]
    )
    return (out1, out2), res


def kernel(seq_1, seq_2):
    return _run(seq_1, seq_2)[0]
